# revision 25
# baseline (speedup 1.0000x reference)
"""VQ codebook (nn_CodeBook) Trainium2 kernel.

Data-parallel over the N axis across 8 NeuronCores; codebook replicated.
Per core (NL = N/8 = 4096 rows):
  1. Normalize codebook rows (wn), store wn to DRAM scratch, and build
     transposed hi/lo bf16 operand tiles wnT[d][128, K] for the PE.
  2. For each m-tile of 128 rows: normalize x rows (xn), transpose to get
     xnT hi/lo bf16, compute dot = xn @ wn.T in 3 bf16 passes
     (hi*hi + hi*lo + lo*hi -> fp32-accuracy), argmax over K via per-chunk
     MAX8 + FIND_INDEX8, gather wn[idx] with indirect DMA, and accumulate
     the squared-error loss partial.
Host sums the 8 per-core loss partials and concatenates outputs.

argmin(dist) == argmax(dot) here because dist = ||xn||^2 + ||wn||^2 - 2 dot
with the row/codeword norm terms ~= 1 (verified exact on the fixed inputs).
"""

import numpy as np

N, D, K = 32768, 512, 8192
NCORES = 8
NL = N // NCORES  # rows per core
P = 128
KC = 1024         # dist chunk width (2 PSUM banks)
COMMITMENT_COST = 0.25

# matmul precision mode:
#   "rescore" = bf16 1-pass screening + exact fp32 top-4 rescore (default)
#   "hilo"    = 3-pass bf16 hi/lo split (fp32-accuracy distances)
#   "fp32"/"fp32r" = native fp32 / reduced-precision 1-pass
MODE = "rescore"

_CACHE = {}


def _build(mode=MODE, nl=NL, k=K, d_dim=D, kc=KC):
    import concourse.bass as bass
    import concourse.tile as tile
    from concourse import bacc, mybir
    from concourse.masks import make_identity

    F32 = mybir.dt.float32
    BF16 = mybir.dt.bfloat16
    I32 = mybir.dt.int32
    U32 = mybir.dt.uint32
    AF = mybir.ActivationFunctionType
    ALU = mybir.AluOpType
    AX = mybir.AxisListType

    MT = nl // P       # m-tiles
    NKC = k // kc      # dist chunks per row
    DT = d_dim // P    # contraction sub-chunks
    WT = k // P        # codebook tiles

    nc = bacc.Bacc("TRN2", target_bir_lowering=False, debug=False)

    x_in = nc.dram_tensor("x", (nl, d_dim), F32, kind="ExternalInput").ap()
    cb_in = nc.dram_tensor("codebook", (k, d_dim), F32, kind="ExternalInput").ap()
    out_q = nc.dram_tensor("out_q", (nl, d_dim), F32, kind="ExternalOutput").ap()
    out_idx = nc.dram_tensor("out_idx", (nl, 1), I32, kind="ExternalOutput").ap()
    out_loss = nc.dram_tensor("out_loss", (P, 1), F32, kind="ExternalOutput").ap()
    wn_dram = nc.dram_tensor("wn_scratch", (k, d_dim), F32).ap()

    hilo = mode == "hilo"
    rescore = mode == "rescore"
    # fp32r tiles must be written as float32r so the producer rounds them
    # to the reduced-precision storage format the PE datapath expects.
    if hilo or rescore:
        mm_dt = BF16
    elif mode == "fp32r":
        mm_dt = mybir.dt.float32r
    else:
        mm_dt = F32

    with tile.TileContext(nc) as tc:
        with (
            tc.tile_pool(name="const", bufs=1) as const_pool,
            tc.tile_pool(name="wnT", bufs=1) as wnT_pool,
            tc.tile_pool(name="wsetup", bufs=3) as wpool,
            tc.tile_pool(name="xio", bufs=4) as xpool,
            tc.tile_pool(name="xn", bufs=4) as xnpool,
            tc.tile_pool(name="xT", bufs=4) as xTpool,
            tc.tile_pool(name="scan", bufs=3) as scanpool,
            tc.tile_pool(name="sdist", bufs=3) as sdistpool,
            tc.tile_pool(name="merge", bufs=3) as mergepool,
            tc.tile_pool(name="qout", bufs=3) as qpool,
            tc.tile_pool(name="acc", bufs=1) as accpool,
            tc.tile_pool(name="psum_mm", bufs=3, space="PSUM") as psum_mm,
            tc.tile_pool(name="psum_tr", bufs=2, space="PSUM") as psum_tr,
        ):
            identity = const_pool.tile([P, P], F32)
            make_identity(nc, identity[:])

            loss_acc = accpool.tile([P, 1], F32)
            nc.vector.memset(loss_acc[:], 0.0)

            # Persistent transposed codebook operand tiles: wnT[d] is
            # [128(d-slice), K]; hi and lo parts for the bf16 split.
            wnT_hi = [wnT_pool.tile([P, k], mm_dt, tag=f"wnT_hi{d}", name=f"wnT_hi{d}") for d in range(DT)]
            wnT_lo = (
                [wnT_pool.tile([P, k], BF16, tag=f"wnT_lo{d}", name=f"wnT_lo{d}") for d in range(DT)]
                if hilo else None
            )

            # ---- codebook setup: normalize + transpose ----
            for t in range(WT):
                w_t = wpool.tile([P, d_dim], F32, tag="w_t")
                nc.sync.dma_start(out=w_t[:], in_=cb_in[t * P:(t + 1) * P, :])

                sq = wpool.tile([P, d_dim], F32, tag="sq")
                ssq = wpool.tile([P, 1], F32, tag="ssq")
                nc.scalar.activation(sq[:], w_t[:], AF.Square, accum_out=ssq[:])
                norm = wpool.tile([P, 1], F32, tag="norm")
                nc.scalar.activation(norm[:], ssq[:], AF.Sqrt)
                rec = wpool.tile([P, 1], F32, tag="rec")
                nc.vector.reciprocal(rec[:], norm[:])
                wn_t = wpool.tile([P, d_dim], F32, tag="wn_t")
                nc.scalar.activation(wn_t[:], w_t[:], AF.Copy, scale=rec[:, :1])
                nc.sync.dma_start(out=wn_dram[t * P:(t + 1) * P, :], in_=wn_t[:])

                # transpose the DT [128,128] blocks -> wnT[d][:, t*128:...]
                tr = psum_tr.tile([P, d_dim], F32, tag="tr", name="tr")
                for d in range(DT):
                    nc.tensor.transpose(
                        out=tr[:, d * P:(d + 1) * P],
                        in_=wn_t[:, d * P:(d + 1) * P],
                        identity=identity[:],
                    )
                ks = slice(t * P, (t + 1) * P)
                for d in range(DT):
                    blk = tr[:, d * P:(d + 1) * P]
                    nc.scalar.activation(wnT_hi[d][:, ks], blk, AF.Copy)
                    if hilo:
                        nc.vector.tensor_tensor(
                            out=wnT_lo[d][:, ks], in0=blk,
                            in1=wnT_hi[d][:, ks], op=ALU.subtract,
                        )

            # ---- main loop over m-tiles ----
            def prep_a(t):
                """Stage A: load x, row sum-of-squares + sqrt (ScalarE)."""
                ms = slice(t * P, (t + 1) * P)
                x_t = xpool.tile([P, d_dim], F32, tag="x_t", name="x_t")
                nc.sync.dma_start(out=x_t[:], in_=x_in[ms, :])
                sqx = xpool.tile([P, d_dim], F32, tag="sqx", name="sqx")
                ssqx = xpool.tile([P, 1], F32, tag="ssqx", name="ssqx")
                nc.scalar.activation(sqx[:], x_t[:], AF.Square, accum_out=ssqx[:])
                normx = xpool.tile([P, 1], F32, tag="normx", name="normx")
                nc.scalar.activation(normx[:], ssqx[:], AF.Sqrt)
                return {"x_t": x_t, "normx": normx}

            def prep(t, sta):
                """Stage B: normalize + build transposed matmul operands."""
                x_t, normx = sta["x_t"], sta["normx"]
                recx = xpool.tile([P, 1], F32, tag="recx", name="recx")
                nc.vector.reciprocal(recx[:], normx[:])
                xn_t = xnpool.tile([P, d_dim], F32, tag="xn_t", name="xn_t")
                nc.scalar.activation(xn_t[:], x_t[:], AF.Copy, scale=recx[:, :1])
                trx = psum_tr.tile([P, d_dim], F32, tag="tr", name="trx")
                for d in range(DT):
                    nc.tensor.transpose(
                        out=trx[:, d * P:(d + 1) * P],
                        in_=xn_t[:, d * P:(d + 1) * P],
                        identity=identity[:],
                    )
                xnT_hi = xTpool.tile([P, d_dim], mm_dt, tag="xnT_hi", name="xnT_hi")
                xnT_lo = (
                    xTpool.tile([P, d_dim], BF16, tag="xnT_lo", name="xnT_lo")
                    if hilo else None
                )
                for d in range(DT):
                    blk = trx[:, d * P:(d + 1) * P]
                    dsl = slice(d * P, (d + 1) * P)
                    nc.scalar.activation(xnT_hi[:, dsl], blk, AF.Copy)
                    if hilo:
                        nc.vector.tensor_tensor(
                            out=xnT_lo[:, dsl], in0=blk,
                            in1=xnT_hi[:, dsl], op=ALU.subtract,
                        )
                return {"xn_t": xn_t, "xnT_hi": xnT_hi, "xnT_lo": xnT_lo}

            def emit_dist_chunks(st, scan_tail):
                """Matmul the dist chunks; call scan_tail(c, dist) per chunk."""
                xnT_hi, xnT_lo = st["xnT_hi"], st["xnT_lo"]
                for c in range(NKC):
                    dist = psum_mm.tile([P, kc], F32, tag="dist", name="dist")
                    for sub in range(kc // 512):
                        kbase = c * kc + sub * 512
                        osl = slice(sub * 512, (sub + 1) * 512)
                        rsl = slice(kbase, kbase + 512)
                        if hilo:
                            seq = []
                            for d in range(DT):
                                seq.append((xnT_hi, wnT_hi[d], d))
                                seq.append((xnT_hi, wnT_lo[d], d))
                                seq.append((xnT_lo, wnT_hi[d], d))
                        else:
                            seq = [(xnT_hi, wnT_hi[d], d) for d in range(DT)]
                        for i, (lhsrc, rsrc, d) in enumerate(seq):
                            nc.tensor.matmul(
                                out=dist[:, osl],
                                lhsT=lhsrc[:, d * P:(d + 1) * P],
                                rhs=rsrc[:, rsl],
                                start=(i == 0),
                                stop=(i == len(seq) - 1),
                            )
                    scan_tail(c, dist)

            def distscan(t, st):
                """Dist chunks; ACT copies PSUM pairs to SBUF; DVE scans 2*kc."""
                npair = NKC // 2
                maxbuf = mergepool.tile([P, npair * 8], F32, tag="maxbuf", name="maxbuf")
                combbuf = mergepool.tile([P, npair * 8], U32, tag="combbuf", name="combbuf")
                holder = {}

                def scan_tail(c, dist):
                    half = c % 2
                    if half == 0:
                        holder["sb"] = sdistpool.tile(
                            [P, 2 * kc], F32, tag="sdist", name="sdist")
                    sb = holder["sb"]
                    nc.scalar.activation(sb[:, half * kc:(half + 1) * kc], dist[:], AF.Copy)
                    if half == 0:
                        return
                    p = c // 2
                    mx8 = maxbuf[:, 8 * p:8 * p + 8]
                    ix8 = scanpool.tile([P, 8], U32, tag="ix8", name="ix8")
                    nc.vector.max(out=mx8, in_=sb[:])
                    nc.vector.max_index(out=ix8[:], in_max=mx8, in_values=sb[:])
                    g8 = scanpool.tile([P, 8], U32, tag="g8", name="g8")
                    nc.vector.tensor_scalar(
                        out=g8[:], in0=ix8[:], scalar1=p * 2 * kc, scalar2=None,
                        op0=ALU.add,
                    )
                    cslot = combbuf[:, 8 * p:8 * p + 8]
                    nc.vector.tensor_scalar(
                        out=cslot, in0=mx8.bitcast(U32), scalar1=0xFFFFE000,
                        scalar2=None, op0=ALU.bitwise_and,
                    )
                    nc.vector.tensor_tensor(
                        out=cslot, in0=cslot, in1=g8[:], op=ALU.bitwise_or,
                    )

                emit_dist_chunks(st, scan_tail)
                # global top-8 packed candidates; low 13 bits carry k
                gmax8 = mergepool.tile([P, 8], F32, tag="gmax8", name="gmax8")
                nc.vector.max(out=gmax8[:], in_=combbuf[:].bitcast(F32))
                candidx = mergepool.tile([P, 8], U32, tag="candidx", name="candidx")
                nc.vector.tensor_scalar(
                    out=candidx[:], in0=gmax8[:].bitcast(U32), scalar1=0x1FFF,
                    scalar2=None, op0=ALU.bitwise_and,
                )
                qrs = []
                for r in range(4):
                    q_r = qpool.tile([P, d_dim], F32, tag=f"qr{r}", name=f"qr{r}")
                    nc.gpsimd.indirect_dma_start(
                        out=q_r[:], out_offset=None, in_=wn_dram[:],
                        in_offset=bass.IndirectOffsetOnAxis(
                            ap=candidx[:, r:r + 1], axis=0),
                    )
                    qrs.append(q_r)
                st["candidx"] = candidx
                st["qrs"] = qrs

            def tail(t, st):
                """Exact fp32 rescore of top-4, select winner, outputs + loss."""
                ms = slice(t * P, (t + 1) * P)
                xn_t, candidx, qrs = st["xn_t"], st["candidx"], st["qrs"]
                sc = []
                cf = []
                for r in range(4):
                    q_r = qrs[r]
                    scr = qpool.tile([P, d_dim], F32, tag="scr", name="scr", bufs=6)
                    scr2 = qpool.tile([P, d_dim], F32, tag="junk", name="scr2", bufs=5)
                    dot_r = mergepool.tile([P, 1], F32, tag=f"dot{r}", name=f"dot{r}")
                    nc.vector.tensor_tensor(
                        out=scr[:], in0=q_r[:], in1=xn_t[:], op=ALU.mult)
                    nc.scalar.activation(
                        scr2[:], scr[:], AF.Copy, accum_out=dot_r[:])
                    sc.append(dot_r)
                    c_f = mergepool.tile([P, 1], F32, tag=f"cf{r}", name=f"cf{r}")
                    nc.vector.tensor_copy(c_f[:], candidx[:, r:r + 1])
                    cf.append(c_f)

                # tournament: winner by strictly-greater score (exact fp32)
                def duel(sa, ia, sb, ib, lbl):
                    ub = mergepool.tile([P, 1], F32, tag=f"ub{lbl}", name=f"ub{lbl}")
                    nc.vector.tensor_tensor(out=ub[:], in0=sb[:], in1=sa[:], op=ALU.is_gt)
                    sw = mergepool.tile([P, 1], F32, tag=f"sw{lbl}", name=f"sw{lbl}")
                    nc.vector.tensor_tensor(out=sw[:], in0=sa[:], in1=sb[:], op=ALU.max)
                    dw = mergepool.tile([P, 1], F32, tag=f"dw{lbl}", name=f"dw{lbl}")
                    nc.vector.tensor_tensor(out=dw[:], in0=ib[:], in1=ia[:], op=ALU.subtract)
                    nc.vector.tensor_tensor(out=dw[:], in0=dw[:], in1=ub[:], op=ALU.mult)
                    iw = mergepool.tile([P, 1], F32, tag=f"iw{lbl}", name=f"iw{lbl}")
                    nc.vector.tensor_tensor(out=iw[:], in0=dw[:], in1=ia[:], op=ALU.add)
                    return sw, iw

                s01, i01 = duel(sc[0], cf[0], sc[1], cf[1], "a")
                s23, i23 = duel(sc[2], cf[2], sc[3], cf[3], "b")
                _, idxf = duel(s01, i01, s23, i23, "c")
                idx_i = mergepool.tile([P, 1], I32, tag="idx_i", name="idx_i")
                nc.vector.tensor_copy(idx_i[:], idxf[:])
                nc.sync.dma_start(out=out_idx[ms, :], in_=idx_i[:])

                q_t = qpool.tile([P, d_dim], F32, tag="q_t", name="q_t")
                nc.gpsimd.indirect_dma_start(
                    out=q_t[:], out_offset=None, in_=wn_dram[:],
                    in_offset=bass.IndirectOffsetOnAxis(ap=idx_i[:, :1], axis=0),
                )
                nc.sync.dma_start(out=out_q[ms, :], in_=q_t[:])

                diff = qpool.tile([P, d_dim], F32, tag="diff", name="diff")
                nc.vector.tensor_tensor(out=diff[:], in0=q_t[:], in1=xn_t[:], op=ALU.subtract)
                dsq = qpool.tile([P, d_dim], F32, tag="junk", name="dsq", bufs=5)
                lrow = qpool.tile([P, 1], F32, tag="lrow", name="lrow")
                nc.scalar.activation(dsq[:], diff[:], AF.Square, accum_out=lrow[:])
                nc.vector.tensor_tensor(out=loss_acc[:], in0=loss_acc[:], in1=lrow[:], op=ALU.add)

            if rescore:
                # software pipeline: stats 3 ahead, operands 2 ahead, tail 1 behind
                sta = {}
                states = {}
                for i in range(min(3, MT)):
                    sta[i] = prep_a(i)
                for i in range(min(2, MT)):
                    states[i] = prep(i, sta.pop(i))
                for t in range(MT):
                    distscan(t, states[t])
                    if t + 3 < MT:
                        sta[t + 3] = prep_a(t + 3)
                    if t + 2 < MT:
                        states[t + 2] = prep(t + 2, sta.pop(t + 2))
                    if t >= 1:
                        tail(t - 1, states.pop(t - 1))
                tail(MT - 1, states.pop(MT - 1))
            else:
                for t in range(MT):
                    ms = slice(t * P, (t + 1) * P)
                    st = prep(t, prep_a(t))
                    xn_t = st["xn_t"]

                    maxbuf = mergepool.tile([P, NKC * 8], F32, tag="maxbuf", name="maxbuf")
                    gidxf = mergepool.tile([P, NKC], F32, tag="gidxf", name="gidxf")

                    def scan_tail(c, dist):
                        mx8 = maxbuf[:, 8 * c:8 * c + 8]
                        ix8 = scanpool.tile([P, 8], U32, tag="ix8", name="ix8")
                        nc.vector.max(out=mx8, in_=dist[:])
                        nc.vector.max_index(out=ix8[:], in_max=mx8, in_values=dist[:])
                        nc.vector.tensor_scalar(
                            out=gidxf[:, c:c + 1], in0=ix8[:, 0:1],
                            scalar1=float(c * kc), scalar2=None, op0=ALU.add,
                        )

                    emit_dist_chunks(st, scan_tail)

                    # merge: smallest global index among chunks achieving the max
                    vmax = mergepool.tile([P, 1], F32, tag="vmax", name="vmax")
                    nc.vector.tensor_reduce(vmax[:], maxbuf[:], axis=AX.X, op=ALU.max)
                    candv = maxbuf[:].rearrange("p (c e) -> p c e", e=8)[:, :, 0:1]
                    notm = mergepool.tile([P, NKC], F32, tag="notm", name="notm")
                    nc.vector.tensor_tensor(
                        out=notm[:], in0=candv,
                        in1=vmax[:, 0:1].to_broadcast([P, NKC, 1]), op=ALU.is_lt,
                    )
                    pen = mergepool.tile([P, NKC], F32, tag="pen", name="pen")
                    nc.vector.tensor_scalar(
                        out=pen[:], in0=notm[:], scalar1=float(2 * k), scalar2=None,
                        op0=ALU.mult,
                    )
                    nc.vector.tensor_tensor(out=pen[:], in0=pen[:], in1=gidxf[:], op=ALU.add)
                    idxf = mergepool.tile([P, 1], F32, tag="idxf", name="idxf")
                    nc.vector.tensor_reduce(idxf[:], pen[:], axis=AX.X, op=ALU.min)
                    idx_i = mergepool.tile([P, 1], I32, tag="idx_i", name="idx_i")
                    nc.vector.tensor_copy(idx_i[:], idxf[:])
                    nc.sync.dma_start(out=out_idx[ms, :], in_=idx_i[:])

                    q_t = qpool.tile([P, d_dim], F32, tag="q_t", name="q_t")
                    nc.gpsimd.indirect_dma_start(
                        out=q_t[:], out_offset=None, in_=wn_dram[:],
                        in_offset=bass.IndirectOffsetOnAxis(ap=idx_i[:, :1], axis=0),
                    )
                    nc.sync.dma_start(out=out_q[ms, :], in_=q_t[:])

                    diff = qpool.tile([P, d_dim], F32, tag="diff", name="diff")
                    nc.vector.tensor_tensor(out=diff[:], in0=q_t[:], in1=xn_t[:], op=ALU.subtract)
                    dsq = qpool.tile([P, d_dim], F32, tag="junk", name="dsq", bufs=5)
                    lrow = qpool.tile([P, 1], F32, tag="lrow", name="lrow")
                    nc.scalar.activation(dsq[:], diff[:], AF.Square, accum_out=lrow[:])
                    nc.vector.tensor_tensor(out=loss_acc[:], in0=loss_acc[:], in1=lrow[:], op=ALU.add)
            nc.sync.dma_start(out=out_loss[:, :], in_=loss_acc[:])

    nc.finalize()
    return nc


def _run(x, codebook, trace=False, mode=MODE):
    from concourse.bass_utils import run_bass_kernel_spmd

    key = ("nc", mode)
    if key not in _CACHE:
        _CACHE[key] = _build(mode)
    nc = _CACHE[key]

    x = np.ascontiguousarray(np.asarray(x, dtype=np.float32))
    codebook = np.ascontiguousarray(np.asarray(codebook, dtype=np.float32))
    in_maps = [
        {"x": x[c * NL:(c + 1) * NL], "codebook": codebook}
        for c in range(NCORES)
    ]
    res = run_bass_kernel_spmd(nc, in_maps, core_ids=list(range(NCORES)), trace=trace)
    q = np.concatenate([res.results[c]["out_q"] for c in range(NCORES)], axis=0)
    idx = np.concatenate(
        [res.results[c]["out_idx"][:, 0] for c in range(NCORES)], axis=0
    ).astype(np.int32)
    loss_sum = sum(res.results[c]["out_loss"].sum(dtype=np.float64) for c in range(NCORES))
    loss = np.float32((1.0 + COMMITMENT_COST) * loss_sum / float(N * D))
    return (q, loss, idx), res


def kernel(x, codebook):
    (q, loss, idx), _ = _run(x, codebook)
    return q, loss, idx


# revision 27
# speedup vs baseline: 1.1373x; 1.1373x over previous
"""VQ codebook (nn_CodeBook) Trainium2 kernel.

Data-parallel over the N axis across 8 NeuronCores; codebook replicated.
Per core (NL = N/8 = 4096 rows):
  1. Normalize codebook rows (wn), store wn to DRAM scratch, and build
     transposed hi/lo bf16 operand tiles wnT[d][128, K] for the PE.
  2. For each m-tile of 128 rows: normalize x rows (xn), transpose to get
     xnT hi/lo bf16, compute dot = xn @ wn.T in 3 bf16 passes
     (hi*hi + hi*lo + lo*hi -> fp32-accuracy), argmax over K via per-chunk
     MAX8 + FIND_INDEX8, gather wn[idx] with indirect DMA, and accumulate
     the squared-error loss partial.
Host sums the 8 per-core loss partials and concatenates outputs.

argmin(dist) == argmax(dot) here because dist = ||xn||^2 + ||wn||^2 - 2 dot
with the row/codeword norm terms ~= 1 (verified exact on the fixed inputs).
"""

import numpy as np

N, D, K = 32768, 512, 8192
NCORES = 8
NL = N // NCORES  # rows per core
P = 128
KC = 1024         # dist chunk width (2 PSUM banks)
COMMITMENT_COST = 0.25

# matmul precision mode:
#   "rescore" = bf16 1-pass screening + exact fp32 top-4 rescore (default)
#   "hilo"    = 3-pass bf16 hi/lo split (fp32-accuracy distances)
#   "fp32"/"fp32r" = native fp32 / reduced-precision 1-pass
MODE = "rescore"

_CACHE = {}


def _build(mode=MODE, nl=NL, k=K, d_dim=D, kc=KC):
    import concourse.bass as bass
    import concourse.tile as tile
    from concourse import bacc, mybir
    from concourse.masks import make_identity

    F32 = mybir.dt.float32
    BF16 = mybir.dt.bfloat16
    I32 = mybir.dt.int32
    U32 = mybir.dt.uint32
    AF = mybir.ActivationFunctionType
    ALU = mybir.AluOpType
    AX = mybir.AxisListType

    MT = nl // P       # m-tiles
    NKC = k // kc      # dist chunks per row
    DT = d_dim // P    # contraction sub-chunks
    WT = k // P        # codebook tiles

    nc = bacc.Bacc("TRN2", target_bir_lowering=False, debug=False)

    x_in = nc.dram_tensor("x", (nl, d_dim), F32, kind="ExternalInput").ap()
    cb_in = nc.dram_tensor("codebook", (k, d_dim), F32, kind="ExternalInput").ap()
    out_q = nc.dram_tensor("out_q", (nl, d_dim), F32, kind="ExternalOutput").ap()
    out_idx = nc.dram_tensor("out_idx", (nl, 1), I32, kind="ExternalOutput").ap()
    out_loss = nc.dram_tensor("out_loss", (P, 1), F32, kind="ExternalOutput").ap()
    wn_dram = nc.dram_tensor("wn_scratch", (k, d_dim), F32).ap()

    hilo = mode == "hilo"
    rescore = mode == "rescore"
    # fp32r tiles must be written as float32r so the producer rounds them
    # to the reduced-precision storage format the PE datapath expects.
    if hilo or rescore:
        mm_dt = BF16
    elif mode == "fp32r":
        mm_dt = mybir.dt.float32r
    else:
        mm_dt = F32

    with tile.TileContext(nc) as tc:
        with (
            tc.tile_pool(name="const", bufs=1) as const_pool,
            tc.tile_pool(name="wnT", bufs=1) as wnT_pool,
            tc.tile_pool(name="wsetup", bufs=3) as wpool,
            tc.tile_pool(name="xio", bufs=3) as xpool,
            tc.tile_pool(name="xn", bufs=4) as xnpool,
            tc.tile_pool(name="xT", bufs=4) as xTpool,
            tc.tile_pool(name="scan", bufs=3) as scanpool,
            tc.tile_pool(name="sdist", bufs=4) as sdistpool,
            tc.tile_pool(name="merge", bufs=3) as mergepool,
            tc.tile_pool(name="qout", bufs=3) as qpool,
            tc.tile_pool(name="acc", bufs=1) as accpool,
            tc.tile_pool(name="psum_mm", bufs=3, space="PSUM") as psum_mm,
            tc.tile_pool(name="psum_tr", bufs=2, space="PSUM") as psum_tr,
        ):
            identity = const_pool.tile([P, P], F32)
            make_identity(nc, identity[:])

            loss_acc = accpool.tile([P, 1], F32)
            nc.vector.memset(loss_acc[:], 0.0)

            # Persistent transposed codebook operand tiles: wnT[d] is
            # [128(d-slice), K]; hi and lo parts for the bf16 split.
            wnT_hi = [wnT_pool.tile([P, k], mm_dt, tag=f"wnT_hi{d}", name=f"wnT_hi{d}") for d in range(DT)]
            wnT_lo = (
                [wnT_pool.tile([P, k], BF16, tag=f"wnT_lo{d}", name=f"wnT_lo{d}") for d in range(DT)]
                if hilo else None
            )

            # ---- codebook setup: normalize + transpose ----
            for t in range(WT):
                w_t = wpool.tile([P, d_dim], F32, tag="w_t")
                nc.sync.dma_start(out=w_t[:], in_=cb_in[t * P:(t + 1) * P, :])

                sq = wpool.tile([P, d_dim], F32, tag="sq")
                ssq = wpool.tile([P, 1], F32, tag="ssq")
                nc.scalar.activation(sq[:], w_t[:], AF.Square, accum_out=ssq[:])
                norm = wpool.tile([P, 1], F32, tag="norm")
                nc.scalar.activation(norm[:], ssq[:], AF.Sqrt)
                rec = wpool.tile([P, 1], F32, tag="rec")
                nc.vector.reciprocal(rec[:], norm[:])
                wn_t = wpool.tile([P, d_dim], F32, tag="wn_t")
                nc.scalar.activation(wn_t[:], w_t[:], AF.Copy, scale=rec[:, :1])
                nc.sync.dma_start(out=wn_dram[t * P:(t + 1) * P, :], in_=wn_t[:])

                # transpose the DT [128,128] blocks -> wnT[d][:, t*128:...]
                tr = psum_tr.tile([P, d_dim], F32, tag="tr", name="tr")
                for d in range(DT):
                    nc.tensor.transpose(
                        out=tr[:, d * P:(d + 1) * P],
                        in_=wn_t[:, d * P:(d + 1) * P],
                        identity=identity[:],
                    )
                ks = slice(t * P, (t + 1) * P)
                for d in range(DT):
                    blk = tr[:, d * P:(d + 1) * P]
                    nc.scalar.activation(wnT_hi[d][:, ks], blk, AF.Copy)
                    if hilo:
                        nc.vector.tensor_tensor(
                            out=wnT_lo[d][:, ks], in0=blk,
                            in1=wnT_hi[d][:, ks], op=ALU.subtract,
                        )

            # ---- main loop over m-tiles ----
            def prep(t):
                """Load + normalize x rows, build transposed matmul operands."""
                ms = slice(t * P, (t + 1) * P)
                x_t = xpool.tile([P, d_dim], F32, tag="x_t", name="x_t")
                nc.sync.dma_start(out=x_t[:], in_=x_in[ms, :])
                sqx = xpool.tile([P, d_dim], F32, tag="sqx", name="sqx")
                ssqx = xpool.tile([P, 1], F32, tag="ssqx", name="ssqx")
                nc.scalar.activation(sqx[:], x_t[:], AF.Square, accum_out=ssqx[:])
                normx = xpool.tile([P, 1], F32, tag="normx", name="normx")
                nc.scalar.activation(normx[:], ssqx[:], AF.Sqrt)
                recx = xpool.tile([P, 1], F32, tag="recx", name="recx")
                nc.vector.reciprocal(recx[:], normx[:])
                xn_t = xnpool.tile([P, d_dim], F32, tag="xn_t", name="xn_t")
                nc.scalar.activation(xn_t[:], x_t[:], AF.Copy, scale=recx[:, :1])
                trx = psum_tr.tile([P, d_dim], F32, tag="tr", name="trx")
                for d in range(DT):
                    nc.tensor.transpose(
                        out=trx[:, d * P:(d + 1) * P],
                        in_=xn_t[:, d * P:(d + 1) * P],
                        identity=identity[:],
                    )
                xnT_hi = xTpool.tile([P, d_dim], mm_dt, tag="xnT_hi", name="xnT_hi")
                xnT_lo = (
                    xTpool.tile([P, d_dim], BF16, tag="xnT_lo", name="xnT_lo")
                    if hilo else None
                )
                for d in range(DT):
                    blk = trx[:, d * P:(d + 1) * P]
                    dsl = slice(d * P, (d + 1) * P)
                    nc.scalar.activation(xnT_hi[:, dsl], blk, AF.Copy)
                    if hilo:
                        nc.vector.tensor_tensor(
                            out=xnT_lo[:, dsl], in0=blk,
                            in1=xnT_hi[:, dsl], op=ALU.subtract,
                        )
                return {"xn_t": xn_t, "xnT_hi": xnT_hi, "xnT_lo": xnT_lo}

            def emit_dist_chunks(st, scan_tail):
                """Matmul the dist chunks; call scan_tail(c, dist) per chunk."""
                xnT_hi, xnT_lo = st["xnT_hi"], st["xnT_lo"]
                for c in range(NKC):
                    dist = psum_mm.tile([P, kc], F32, tag="dist", name="dist")
                    for sub in range(kc // 512):
                        kbase = c * kc + sub * 512
                        osl = slice(sub * 512, (sub + 1) * 512)
                        rsl = slice(kbase, kbase + 512)
                        if hilo:
                            seq = []
                            for d in range(DT):
                                seq.append((xnT_hi, wnT_hi[d], d))
                                seq.append((xnT_hi, wnT_lo[d], d))
                                seq.append((xnT_lo, wnT_hi[d], d))
                        else:
                            seq = [(xnT_hi, wnT_hi[d], d) for d in range(DT)]
                        for i, (lhsrc, rsrc, d) in enumerate(seq):
                            nc.tensor.matmul(
                                out=dist[:, osl],
                                lhsT=lhsrc[:, d * P:(d + 1) * P],
                                rhs=rsrc[:, rsl],
                                start=(i == 0),
                                stop=(i == len(seq) - 1),
                            )
                    scan_tail(c, dist)

            def distscan(t, st):
                """Dist chunks; ACT copies PSUM pairs to SBUF; DVE scans 2*kc."""
                npair = NKC // 2
                maxbuf = mergepool.tile([P, npair * 8], F32, tag="maxbuf", name="maxbuf")
                combbuf = mergepool.tile([P, npair * 8], U32, tag="combbuf", name="combbuf")
                holder = {}

                def scan_tail(c, dist):
                    half = c % 2
                    if half == 0:
                        holder["sb"] = sdistpool.tile(
                            [P, 2 * kc], F32, tag="sdist", name="sdist")
                    sb = holder["sb"]
                    nc.scalar.activation(sb[:, half * kc:(half + 1) * kc], dist[:], AF.Copy)
                    if half == 0:
                        return
                    p = c // 2
                    mx8 = maxbuf[:, 8 * p:8 * p + 8]
                    ix8 = scanpool.tile([P, 8], U32, tag="ix8", name="ix8")
                    nc.vector.max(out=mx8, in_=sb[:])
                    nc.vector.max_index(out=ix8[:], in_max=mx8, in_values=sb[:])
                    g8 = scanpool.tile([P, 8], U32, tag="g8", name="g8")
                    nc.vector.tensor_scalar(
                        out=g8[:], in0=ix8[:], scalar1=p * 2 * kc, scalar2=None,
                        op0=ALU.add,
                    )
                    cslot = combbuf[:, 8 * p:8 * p + 8]
                    nc.vector.tensor_scalar(
                        out=cslot, in0=mx8.bitcast(U32), scalar1=0xFFFFE000,
                        scalar2=None, op0=ALU.bitwise_and,
                    )
                    nc.vector.tensor_tensor(
                        out=cslot, in0=cslot, in1=g8[:], op=ALU.bitwise_or,
                    )

                emit_dist_chunks(st, scan_tail)
                # global top-8 packed candidates; low 13 bits carry k
                gmax8 = mergepool.tile([P, 8], F32, tag="gmax8", name="gmax8")
                nc.vector.max(out=gmax8[:], in_=combbuf[:].bitcast(F32))
                candidx = mergepool.tile([P, 8], U32, tag="candidx", name="candidx")
                nc.vector.tensor_scalar(
                    out=candidx[:], in0=gmax8[:].bitcast(U32), scalar1=0x1FFF,
                    scalar2=None, op0=ALU.bitwise_and,
                )
                qrs = []
                for r in range(4):
                    q_r = qpool.tile([P, d_dim], F32, tag=f"qr{r}", name=f"qr{r}")
                    nc.gpsimd.indirect_dma_start(
                        out=q_r[:], out_offset=None, in_=wn_dram[:],
                        in_offset=bass.IndirectOffsetOnAxis(
                            ap=candidx[:, r:r + 1], axis=0),
                    )
                    qrs.append(q_r)
                st["candidx"] = candidx
                st["qrs"] = qrs

            def tail(t, st):
                """Exact fp32 rescore of top-4, select winner, outputs + loss."""
                ms = slice(t * P, (t + 1) * P)
                xn_t, candidx, qrs = st["xn_t"], st["candidx"], st["qrs"]
                sc = []
                cf = []
                for r in range(4):
                    q_r = qrs[r]
                    scr = qpool.tile([P, d_dim], F32, tag="scr", name="scr")
                    scr2 = qpool.tile([P, d_dim], F32, tag="junk", name="scr2")
                    dot_r = mergepool.tile([P, 1], F32, tag=f"dot{r}", name=f"dot{r}")
                    nc.vector.tensor_tensor(
                        out=scr[:], in0=q_r[:], in1=xn_t[:], op=ALU.mult)
                    nc.scalar.activation(
                        scr2[:], scr[:], AF.Copy, accum_out=dot_r[:])
                    sc.append(dot_r)
                    c_f = mergepool.tile([P, 1], F32, tag=f"cf{r}", name=f"cf{r}")
                    nc.vector.tensor_copy(c_f[:], candidx[:, r:r + 1])
                    cf.append(c_f)

                # tournament: winner by strictly-greater score (exact fp32)
                def duel(sa, ia, sb, ib, lbl):
                    ub = mergepool.tile([P, 1], F32, tag=f"ub{lbl}", name=f"ub{lbl}")
                    nc.vector.tensor_tensor(out=ub[:], in0=sb[:], in1=sa[:], op=ALU.is_gt)
                    sw = mergepool.tile([P, 1], F32, tag=f"sw{lbl}", name=f"sw{lbl}")
                    nc.vector.tensor_tensor(out=sw[:], in0=sa[:], in1=sb[:], op=ALU.max)
                    dw = mergepool.tile([P, 1], F32, tag=f"dw{lbl}", name=f"dw{lbl}")
                    nc.vector.tensor_tensor(out=dw[:], in0=ib[:], in1=ia[:], op=ALU.subtract)
                    nc.vector.tensor_tensor(out=dw[:], in0=dw[:], in1=ub[:], op=ALU.mult)
                    iw = mergepool.tile([P, 1], F32, tag=f"iw{lbl}", name=f"iw{lbl}")
                    nc.vector.tensor_tensor(out=iw[:], in0=dw[:], in1=ia[:], op=ALU.add)
                    return sw, iw

                s01, i01 = duel(sc[0], cf[0], sc[1], cf[1], "a")
                s23, i23 = duel(sc[2], cf[2], sc[3], cf[3], "b")
                _, idxf = duel(s01, i01, s23, i23, "c")
                idx_i = mergepool.tile([P, 1], I32, tag="idx_i", name="idx_i")
                nc.vector.tensor_copy(idx_i[:], idxf[:])
                nc.sync.dma_start(out=out_idx[ms, :], in_=idx_i[:])

                q_t = qpool.tile([P, d_dim], F32, tag="q_t", name="q_t")
                nc.gpsimd.indirect_dma_start(
                    out=q_t[:], out_offset=None, in_=wn_dram[:],
                    in_offset=bass.IndirectOffsetOnAxis(ap=idx_i[:, :1], axis=0),
                )
                nc.sync.dma_start(out=out_q[ms, :], in_=q_t[:])

                diff = qpool.tile([P, d_dim], F32, tag="diff", name="diff")
                nc.vector.tensor_tensor(out=diff[:], in0=q_t[:], in1=xn_t[:], op=ALU.subtract)
                dsq = qpool.tile([P, d_dim], F32, tag="junk", name="dsq")
                lrow = qpool.tile([P, 1], F32, tag="lrow", name="lrow")
                nc.scalar.activation(dsq[:], diff[:], AF.Square, accum_out=lrow[:])
                nc.vector.tensor_tensor(out=loss_acc[:], in0=loss_acc[:], in1=lrow[:], op=ALU.add)

            if rescore:
                # software pipeline: prep 2 ahead, rescore-tail 1 behind
                states = {}
                states[0] = prep(0)
                if MT > 1:
                    states[1] = prep(1)
                for t in range(MT):
                    distscan(t, states[t])
                    if t >= 1:
                        tail(t - 1, states.pop(t - 1))
                    if t + 2 < MT:
                        states[t + 2] = prep(t + 2)
                tail(MT - 1, states.pop(MT - 1))
            else:
                for t in range(MT):
                    ms = slice(t * P, (t + 1) * P)
                    st = prep(t)
                    xn_t = st["xn_t"]

                    maxbuf = mergepool.tile([P, NKC * 8], F32, tag="maxbuf", name="maxbuf")
                    gidxf = mergepool.tile([P, NKC], F32, tag="gidxf", name="gidxf")

                    def scan_tail(c, dist):
                        mx8 = maxbuf[:, 8 * c:8 * c + 8]
                        ix8 = scanpool.tile([P, 8], U32, tag="ix8", name="ix8")
                        nc.vector.max(out=mx8, in_=dist[:])
                        nc.vector.max_index(out=ix8[:], in_max=mx8, in_values=dist[:])
                        nc.vector.tensor_scalar(
                            out=gidxf[:, c:c + 1], in0=ix8[:, 0:1],
                            scalar1=float(c * kc), scalar2=None, op0=ALU.add,
                        )

                    emit_dist_chunks(st, scan_tail)

                    # merge: smallest global index among chunks achieving the max
                    vmax = mergepool.tile([P, 1], F32, tag="vmax", name="vmax")
                    nc.vector.tensor_reduce(vmax[:], maxbuf[:], axis=AX.X, op=ALU.max)
                    candv = maxbuf[:].rearrange("p (c e) -> p c e", e=8)[:, :, 0:1]
                    notm = mergepool.tile([P, NKC], F32, tag="notm", name="notm")
                    nc.vector.tensor_tensor(
                        out=notm[:], in0=candv,
                        in1=vmax[:, 0:1].to_broadcast([P, NKC, 1]), op=ALU.is_lt,
                    )
                    pen = mergepool.tile([P, NKC], F32, tag="pen", name="pen")
                    nc.vector.tensor_scalar(
                        out=pen[:], in0=notm[:], scalar1=float(2 * k), scalar2=None,
                        op0=ALU.mult,
                    )
                    nc.vector.tensor_tensor(out=pen[:], in0=pen[:], in1=gidxf[:], op=ALU.add)
                    idxf = mergepool.tile([P, 1], F32, tag="idxf", name="idxf")
                    nc.vector.tensor_reduce(idxf[:], pen[:], axis=AX.X, op=ALU.min)
                    idx_i = mergepool.tile([P, 1], I32, tag="idx_i", name="idx_i")
                    nc.vector.tensor_copy(idx_i[:], idxf[:])
                    nc.sync.dma_start(out=out_idx[ms, :], in_=idx_i[:])

                    q_t = qpool.tile([P, d_dim], F32, tag="q_t", name="q_t")
                    nc.gpsimd.indirect_dma_start(
                        out=q_t[:], out_offset=None, in_=wn_dram[:],
                        in_offset=bass.IndirectOffsetOnAxis(ap=idx_i[:, :1], axis=0),
                    )
                    nc.sync.dma_start(out=out_q[ms, :], in_=q_t[:])

                    diff = qpool.tile([P, d_dim], F32, tag="diff", name="diff")
                    nc.vector.tensor_tensor(out=diff[:], in0=q_t[:], in1=xn_t[:], op=ALU.subtract)
                    dsq = qpool.tile([P, d_dim], F32, tag="junk", name="dsq")
                    lrow = qpool.tile([P, 1], F32, tag="lrow", name="lrow")
                    nc.scalar.activation(dsq[:], diff[:], AF.Square, accum_out=lrow[:])
                    nc.vector.tensor_tensor(out=loss_acc[:], in0=loss_acc[:], in1=lrow[:], op=ALU.add)
            nc.sync.dma_start(out=out_loss[:, :], in_=loss_acc[:])

    nc.finalize()
    return nc


def _run(x, codebook, trace=False, mode=MODE):
    from concourse.bass_utils import run_bass_kernel_spmd

    key = ("nc", mode)
    if key not in _CACHE:
        _CACHE[key] = _build(mode)
    nc = _CACHE[key]

    x = np.ascontiguousarray(np.asarray(x, dtype=np.float32))
    codebook = np.ascontiguousarray(np.asarray(codebook, dtype=np.float32))
    in_maps = [
        {"x": x[c * NL:(c + 1) * NL], "codebook": codebook}
        for c in range(NCORES)
    ]
    res = run_bass_kernel_spmd(nc, in_maps, core_ids=list(range(NCORES)), trace=trace)
    q = np.concatenate([res.results[c]["out_q"] for c in range(NCORES)], axis=0)
    idx = np.concatenate(
        [res.results[c]["out_idx"][:, 0] for c in range(NCORES)], axis=0
    ).astype(np.int32)
    loss_sum = sum(res.results[c]["out_loss"].sum(dtype=np.float64) for c in range(NCORES))
    loss = np.float32((1.0 + COMMITMENT_COST) * loss_sum / float(N * D))
    return (q, loss, idx), res


def kernel(x, codebook):
    (q, loss, idx), _ = _run(x, codebook)
    return q, loss, idx
